# revision 7
# baseline (speedup 1.0000x reference)
"""Multi-head attention Trainium2 kernel.

B=4, S=1024, D=1024, H=16, hd=64, f32 reference. 8 NeuronCores:
core c handles batch b=c//2, head-group g=c%2 (8 heads each) —
tensor-parallel over heads within a batch; the host sums the two
partial output projections per batch (the "all-reduce" of the
sharding hint) and adds bo.

Device dataflow (per core), everything feature-major so there are no
on-device transposes:
  qT[c,s] = sum_i Wq[i,c] xT[i,s] + bq          (lhsT=Wq tile, rhs=xT)
  kT      = (k_raw + bk) * 0.125                (1/sqrt(hd) folded in)
  V[s,c]  = sum_i xT[i,s] Wv[i,c] + bv          (token-major; Wv is
            augmented with a zero column + bias 1.0 per head, giving a
            ones column in V => softmax denominator falls out of the
            PV matmul as row 64)
  ST[k,q] = kT.T @ qT          (scores transposed, 2 heads row-tiled)
  ST     += maskTneg           (mask==0 -> -8e4; exp underflows to 0;
                                softmax max-subtraction is unnecessary:
                                scaled scores are ~N(0,1))
  PT      = exp(ST)
  valsT_aug[65,q] = sum over k-tiles of V_aug.T-ish matmul
                    (lhsT=V_aug[k,65], rhs=PT[k,q])
  vals    = valsT * (1/denom)  (reciprocal_approx_accurate + K=1 ones
                                broadcast matmul, fused into the
                                PSUM->SBUF copy)
  out_partial[q,n] = vals.T @ Wo_rows
All matmuls are float32r (full PE rate at N>=512, ~1.5e-4 rounding).
"""

import numpy as np

import concourse.bacc as bacc
import concourse.mybir as mybir
import concourse.tile as tile
from concourse import bass_utils
from concourse.alu_op_type import AluOpType

F32 = mybir.dt.float32
F32R = mybir.dt.float32r
I32 = mybir.dt.int32
AF = mybir.ActivationFunctionType

B, S, D, H, HD = 4, 1024, 1024, 16, 64
NCORES = 8
HPC = 8            # heads per core
HAUG = HD + 1      # 65: V columns per head incl. ones column
VW = HPC * HAUG    # 520
NEG = -80000.0     # mask fill; exp(0.125 * -80000) == 0 in f32


def build_kernel(debug=False):
    nc = bacc.Bacc(trn_type="TRN2", target_bir_lowering=False, debug=False,
                   num_devices=NCORES)

    xT = nc.dram_tensor("xT", [D, S], F32R, kind="ExternalInput").ap()
    maskT = nc.dram_tensor("maskT", [S, S], I32, kind="ExternalInput").ap()
    wq = nc.dram_tensor("wq", [D, 512], F32R, kind="ExternalInput").ap()
    wk = nc.dram_tensor("wk", [D, 512], F32R, kind="ExternalInput").ap()
    wv = nc.dram_tensor("wv", [D, VW], F32R, kind="ExternalInput").ap()
    bq = nc.dram_tensor("bq", [512], F32, kind="ExternalInput").ap()
    bk = nc.dram_tensor("bk", [512], F32, kind="ExternalInput").ap()
    bv = nc.dram_tensor("bv", [VW], F32R, kind="ExternalInput").ap()
    wo = nc.dram_tensor("wo", [512, S], F32R, kind="ExternalInput").ap()
    onesd = nc.dram_tensor("onesd", [1, 128], F32R, kind="ExternalInput").ap()
    out = nc.dram_tensor("out", [S, S], F32, kind="ExternalOutput").ap()
    if debug:
        d_q = nc.dram_tensor("d_q", [512, S], F32, kind="ExternalOutput").ap()
        d_k = nc.dram_tensor("d_k", [512, S], F32, kind="ExternalOutput").ap()
        d_v = nc.dram_tensor("d_v", [8 * 128, VW], F32, kind="ExternalOutput").ap()
        d_m = nc.dram_tensor("d_m", [S, S], F32, kind="ExternalOutput").ap()
        d_vals = nc.dram_tensor("d_vals", [512, S], F32, kind="ExternalOutput").ap()
        d_vps = nc.dram_tensor("d_vps", [HAUG, S], F32, kind="ExternalOutput").ap()
        d_rec = nc.dram_tensor("d_rec", [1, S], F32, kind="ExternalOutput").ap()
        d_rc = nc.dram_tensor("d_rc", [1, S], F32, kind="ExternalOutput").ap()
        d_bcs = nc.dram_tensor("d_bcs", [64, S], F32, kind="ExternalOutput").ap()

    with tile.TileContext(nc) as tc:
        with (
            tc.tile_pool(name="persist", bufs=1) as P,
            tc.tile_pool(name="psum", bufs=1, space="PSUM") as PP,
        ):
            # ---- persistent sbuf ----
            qT = [P.tile([128, S], F32R, tag=f"qT{t}", name=f"qT{t}") for t in range(4)]
            kT = [P.tile([128, S], F32R, tag=f"kT{t}", name=f"kT{t}") for t in range(4)]
            vA = [P.tile([128, VW], F32R, tag=f"vA{t}", name=f"vA{t}") for t in range(8)]
            mN = [P.tile([128, S], F32, tag=f"mN{t}", name=f"mN{t}") for t in range(8)]
            bq_t = P.tile([128, 4], F32, tag="bq", name="bq")
            bk_t = P.tile([128, 4], F32, tag="bk", name="bk")
            bv_row = P.tile([1, VW], F32R, tag="bvrow", name="bvrow")
            ones_row = P.tile([1, 128], F32R, tag="ones", name="ones")
            bvb = P.tile([128, VW], F32, tag="bvb", name="bvb")

            nc.sync.dma_start(bq_t[:], bq.rearrange("(t p) -> p t", p=128))
            nc.sync.dma_start(bk_t[:], bk.rearrange("(t p) -> p t", p=128))
            nc.sync.dma_start(bv_row[:], bv[None, :])
            nc.sync.dma_start(ones_row[:], onesd)

            # broadcast bv across 128 partitions via K=1 matmul
            for h2 in range(2):
                cs = slice(h2 * 260, (h2 + 1) * 260)
                pb = PP.tile([128, 260], F32, tag=f"v{h2}", name=f"v{h2}")
                nc.tensor.matmul(pb[:], ones_row[:, 0:128], bv_row[:, cs],
                                 start=True, stop=True)
                nc.scalar.activation(bvb[:, cs], pb[:], AF.Identity)

            # ---- stage 1: QKV projections ----
            with tc.tile_pool(name="s1", bufs=1) as S1:
                xT_t = [S1.tile([128, S], F32R, tag=f"xT{i}", name=f"xT{i}") for i in range(8)]
                wq_t = [S1.tile([128, 512], F32R, tag=f"wq{i}", name=f"wq{i}") for i in range(8)]
                wk_t = [S1.tile([128, 512], F32R, tag=f"wk{i}", name=f"wk{i}") for i in range(8)]
                wv_t = [S1.tile([128, VW], F32R, tag=f"wv{i}", name=f"wv{i}") for i in range(8)]
                for i in range(8):
                    rs = slice(i * 128, (i + 1) * 128)
                    nc.sync.dma_start(xT_t[i][:], xT[rs, :])
                    nc.sync.dma_start(wq_t[i][:], wq[rs, :])
                    nc.sync.dma_start(wk_t[i][:], wk[rs, :])
                    nc.sync.dma_start(wv_t[i][:], wv[rs, :])

                # q/k feature-major: psum[c,s] accumulated over i-tiles
                for t in range(4):
                    cs = slice(t * 128, (t + 1) * 128)
                    for sh in range(2):
                        ss = slice(sh * 512, (sh + 1) * 512)
                        pq = PP.tile([128, 512], F32, tag=f"st{2 * sh + (t & 1)}", name=f"st{2 * sh + (t & 1)}")
                        pk = PP.tile([128, 512], F32, tag=f"v{2 * sh + (t & 1)}", name=f"v{2 * sh + (t & 1)}")
                        for i in range(8):
                            nc.tensor.matmul(pq[:], wq_t[i][:, cs],
                                             xT_t[i][:, ss],
                                             start=(i == 0), stop=(i == 7))
                        for i in range(8):
                            nc.tensor.matmul(pk[:], wk_t[i][:, cs],
                                             xT_t[i][:, ss],
                                             start=(i == 0), stop=(i == 7))
                        # qT = pq + bq (ACT), kT = (pk + bk)*0.125 (DVE)
                        nc.scalar.activation(qT[t][:, ss], pq[:], AF.Identity,
                                             bias=bq_t[:, t:t + 1])
                        with nc.allow_low_precision(reason="f32r matmul feed"):
                            nc.vector.tensor_scalar(kT[t][:, ss], pk[:],
                                                    bk_t[:, t:t + 1], 0.125,
                                                    AluOpType.add,
                                                    AluOpType.mult)

                # V token-major with augmented ones column
                for st in range(8):
                    ts_ = slice(st * 128, (st + 1) * 128)
                    for h2 in range(2):
                        cs = slice(h2 * 260, (h2 + 1) * 260)
                        pv = PP.tile([128, 260], F32,
                                     tag=f"v{2 * h2 + (st & 1)}",
                                     name=f"pv{2 * h2 + (st & 1)}")
                        for i in range(8):
                            nc.tensor.matmul(pv[:], xT_t[i][:, ts_],
                                             wv_t[i][:, cs],
                                             start=(i == 0), stop=(i == 7))
                        with nc.allow_low_precision(reason="f32r matmul feed"):
                            nc.vector.tensor_tensor(vA[st][:, cs], pv[:],
                                                    bvb[:, cs], AluOpType.add)

            # ---- stage 1.5: mask -> f32 additive ----
            nbias = P.tile([128, 1], F32, tag="nbias", name="nbias")
            nc.vector.memset(nbias[:], NEG)
            with tc.tile_pool(name="sm", bufs=2) as SM:
                for i in range(8):
                    mi = SM.tile([128, S], I32, tag="mi", name="mi")
                    nc.sync.dma_start(mi[:], maskT[i * 128:(i + 1) * 128, :])
                    # (m - 1) * 8e4 : 0 where mask==1, -8e4 where mask==0
                    nc.scalar.activation(mN[i][:], mi[:], AF.Identity,
                                         bias=nbias[:], scale=-NEG)

            if debug:
                for t in range(4):
                    nc.sync.dma_start(d_q[t * 128:(t + 1) * 128, :], qT[t][:].bitcast(F32))
                    nc.sync.dma_start(d_k[t * 128:(t + 1) * 128, :], kT[t][:].bitcast(F32))
                for t in range(8):
                    nc.sync.dma_start(d_v[t * 128:(t + 1) * 128, :], vA[t][:].bitcast(F32))
                    nc.sync.dma_start(d_m[t * 128:(t + 1) * 128, :], mN[t][:])

            # ---- stage 2+3: attention + output projection ----
            with tc.tile_pool(name="sao", bufs=1) as SA:
                vals = [SA.tile([128, S], F32R, tag=f"vals{p}", name=f"vals{p}")
                        for p in range(4)]
                wo_t = [SA.tile([128, S], F32R, tag=f"wo{t}", name=f"wo{t}") for t in range(4)]
                for t in range(4):
                    nc.sync.dma_start(wo_t[t][:], wo[t * 128:(t + 1) * 128, :])

                for p in range(4):   # head pair
                    vps = [[PP.tile([HAUG, 512], F32, tag=f"v{2 * hh + qh}", name=f"v{2 * hh + qh}")
                            for qh in range(2)] for hh in range(2)]
                    for kt in range(8):
                        ks = slice(kt * 128, (kt + 1) * 128)
                        for qh in range(2):
                            qs = slice(qh * 512, (qh + 1) * 512)
                            for hh in range(2):
                                h = 2 * p + hh
                                ds = slice(hh * 64, (hh + 1) * 64)
                                stp = PP.tile([128, 512], F32,
                                              tag=f"st{2 * hh + qh}",
                                              name=f"stp{2 * hh + qh}")
                                nc.tensor.matmul(stp[:], kT[p][ds, ks],
                                                 qT[p][ds, qs],
                                                 start=True, stop=True,
                                                 tile_position=(hh * 64, 0))
                                nc.vector.tensor_tensor(stp[:], stp[:],
                                                        mN[kt][:, qs],
                                                        AluOpType.add)
                                pt = SA.tile([128, 512], F32R, tag="pt", name="pt")
                                nc.scalar.activation(pt[:], stp[:], AF.Exp)
                                nc.tensor.matmul(
                                    vps[hh][qh][:],
                                    vA[kt][:, h * HAUG:(h + 1) * HAUG],
                                    pt[:],
                                    start=(kt == 0), stop=(kt == 7))
                    if debug and p == 3:
                        for qh in range(2):
                            dvs = SA.tile([HAUG, 512], F32, tag="dvs", name="dvs")
                            nc.scalar.activation(dvs[:], vps[0][qh][:], AF.Identity)
                            nc.sync.dma_start(d_vps[:, qh * 512:(qh + 1) * 512], dvs[:])
                    # normalize: vals[p][hh*64:(hh+1)*64] = valsT / denom
                    for hh in range(2):
                        den = SA.tile([1, S], F32, tag="den", name="den")
                        rec = SA.tile([1, S], F32, tag="rec", name="rec")
                        scr = SA.tile([1, S], F32, tag="scr", name="scr")
                        rc = SA.tile([1, S], F32R, tag="rc", name="rc")
                        for qh in range(2):
                            qs = slice(qh * 512, (qh + 1) * 512)
                            nc.scalar.activation(den[:, qs],
                                                 vps[hh][qh][64:65, :],
                                                 AF.Identity)
                        nc.vector.reciprocal_approx_accurate(
                            rec[:], den[:], scr[:])
                        with nc.allow_low_precision(reason="f32r matmul feed"):
                            nc.vector.tensor_copy(rc[:], rec[:])
                        if debug and p == 3 and hh == 0:
                            nc.sync.dma_start(d_rec, rec[:])
                            nc.sync.dma_start(d_rc, rc[:].bitcast(F32))
                        for qh in range(2):
                            qs = slice(qh * 512, (qh + 1) * 512)
                            bcp = PP.tile([64, 512], F32, tag=f"st{2 * hh}", name=f"st{2 * hh}")
                            nc.tensor.matmul(bcp[:], ones_row[:, 0:64],
                                             rc[:, qs], start=True, stop=True)
                            bcs = SA.tile([64, 512], F32, tag="bcs", name="bcs")
                            nc.scalar.activation(bcs[:], bcp[:], AF.Identity)
                            if debug and p == 3 and hh == 0:
                                nc.sync.dma_start(
                                    d_bcs[:, qh * 512:(qh + 1) * 512], bcs[:])
                            with nc.allow_low_precision(reason="f32r feed"):
                                nc.vector.tensor_tensor(
                                    vals[p][hh * 64:(hh + 1) * 64, qs],
                                    vps[hh][qh][0:64, :], bcs[:],
                                    AluOpType.mult)

                if debug:
                    for pi in range(4):
                        nc.sync.dma_start(d_vals[pi * 128:(pi + 1) * 128, :],
                                          vals[pi][:].bitcast(F32))

                # output projection: out[q,n] = vals.T @ wo
                for qt in range(8):
                    qs = slice(qt * 128, (qt + 1) * 128)
                    for nh in range(2):
                        ns = slice(nh * 512, (nh + 1) * 512)
                        po = PP.tile([128, 512], F32,
                                     tag=f"st{2 * nh + (qt & 1)}",
                                     name=f"po{2 * nh + (qt & 1)}")
                        for pi in range(4):
                            nc.tensor.matmul(po[:], vals[pi][:, qs],
                                             wo_t[pi][:, ns],
                                             start=(pi == 0), stop=(pi == 3))
                        ot = SA.tile([128, 512], F32, tag="ot", name="ot")
                        nc.scalar.activation(ot[:], po[:], AF.Identity)
                        nc.sync.dma_start(out[qs, ns], ot[:])

    nc.compile()
    return nc


_NC_CACHE = {}


def _get_nc():
    if "nc" not in _NC_CACHE:
        _NC_CACHE["nc"] = build_kernel()
    return _NC_CACHE["nc"]


def shard_inputs(x, mask, Wqkv, bqkv, Wo, bo):
    """Per-core input dicts. Layout/slicing only — no arithmetic."""
    x = np.ascontiguousarray(np.asarray(x, dtype=np.float32))
    mask = np.ascontiguousarray(np.asarray(mask, dtype=np.int32))
    Wqkv = np.asarray(Wqkv, dtype=np.float32)
    bqkv = np.asarray(bqkv, dtype=np.float32)
    Wo = np.asarray(Wo, dtype=np.float32)

    Wr = Wqkv.reshape(D, H, 3, HD)
    br = bqkv.reshape(H, 3, HD)
    ones = np.ones((1, 128), dtype=np.float32)
    in_maps = []
    for c in range(NCORES):
        b, g = c // 2, c % 2
        hs = slice(g * HPC, (g + 1) * HPC)
        wv_aug = np.zeros((D, HPC, HAUG), dtype=np.float32)
        wv_aug[:, :, :HD] = Wr[:, hs, 2, :]
        bv_aug = np.zeros((HPC, HAUG), dtype=np.float32)
        bv_aug[:, :HD] = br[hs, 2, :]
        bv_aug[:, HD] = 1.0
        in_maps.append({
            "xT": np.ascontiguousarray(x[b].T),
            "maskT": np.ascontiguousarray(mask[b].T),
            "wq": np.ascontiguousarray(Wr[:, hs, 0, :].reshape(D, 512)),
            "wk": np.ascontiguousarray(Wr[:, hs, 1, :].reshape(D, 512)),
            "wv": np.ascontiguousarray(wv_aug.reshape(D, VW)),
            "bq": np.ascontiguousarray(br[hs, 0, :].reshape(512)),
            "bk": np.ascontiguousarray(br[hs, 1, :].reshape(512)),
            "bv": np.ascontiguousarray(bv_aug.reshape(VW)),
            "wo": np.ascontiguousarray(Wo[g * 512:(g + 1) * 512, :]),
            "onesd": ones,
        })
    return in_maps


def combine_outputs(results, bo):
    bo = np.asarray(bo, dtype=np.float32)
    out = np.empty((B, S, D), dtype=np.float32)
    for b in range(B):
        out[b] = results[2 * b]["out"] + results[2 * b + 1]["out"] + bo
    return out


def kernel(x, mask, Wqkv, bqkv, Wo, bo):
    nc = _get_nc()
    in_maps = shard_inputs(x, mask, Wqkv, bqkv, Wo, bo)
    res = bass_utils.run_bass_kernel_spmd(nc, in_maps,
                                          core_ids=list(range(NCORES)))
    return combine_outputs(res.results, bo)


# revision 8
# speedup vs baseline: 72.4978x; 72.4978x over previous
"""Multi-head attention Trainium2 kernel.

B=4, S=1024, D=1024, H=16, hd=64, f32 reference. 8 NeuronCores:
core c handles batch b=c//2, head-group g=c%2 (8 heads each) —
tensor-parallel over heads within a batch; the host sums the two
partial output projections per batch (the "all-reduce" of the
sharding hint) and adds bo.

Device dataflow (per core), everything feature-major so there are no
on-device transposes:
  qT[c,s] = sum_i Wq[i,c] xT[i,s] + bq          (lhsT=Wq tile, rhs=xT)
  kT      = (k_raw + bk) * 0.125                (1/sqrt(hd) folded in)
  V[s,c]  = sum_i xT[i,s] Wv[i,c] + bv          (token-major; Wv is
            augmented with a zero column + bias 1.0 per head, giving a
            ones column in V => softmax denominator falls out of the
            PV matmul as row 64)
  ST[k,q] = kT.T @ qT          (scores transposed, 2 heads row-tiled)
  ST     += maskTneg           (mask==0 -> -8e4; exp underflows to 0;
                                softmax max-subtraction is unnecessary:
                                scaled scores are ~N(0,1))
  PT      = exp(ST)
  valsT_aug[65,q] = sum over k-tiles of V_aug.T-ish matmul
                    (lhsT=V_aug[k,65], rhs=PT[k,q])
  vals    = valsT * (1/denom)  (reciprocal_approx_accurate + K=1 ones
                                broadcast matmul, fused into the
                                PSUM->SBUF copy)
  out_partial[q,n] = vals.T @ Wo_rows
All matmuls are float32r (full PE rate at N>=512, ~1.5e-4 rounding).
"""

import numpy as np

import concourse.bacc as bacc
import concourse.mybir as mybir
import concourse.tile as tile
from concourse import bass_utils
from concourse.alu_op_type import AluOpType

F32 = mybir.dt.float32
F32R = mybir.dt.float32r
I32 = mybir.dt.int32
AF = mybir.ActivationFunctionType

B, S, D, H, HD = 4, 1024, 1024, 16, 64
NCORES = 8
HPC = 8            # heads per core
HAUG = HD + 1      # 65: V columns per head incl. ones column
VW = HPC * HAUG    # 520
NEG = -80000.0     # mask fill; exp(0.125 * -80000) == 0 in f32


def build_kernel(debug=False, krep=1):
    nc = bacc.Bacc(trn_type="TRN2", target_bir_lowering=False, debug=False,
                   num_devices=NCORES)

    xT = nc.dram_tensor("xT", [D, S], F32R, kind="ExternalInput").ap()
    maskT = nc.dram_tensor("maskT", [S, S], I32, kind="ExternalInput").ap()
    wq = nc.dram_tensor("wq", [D, 512], F32R, kind="ExternalInput").ap()
    wk = nc.dram_tensor("wk", [D, 512], F32R, kind="ExternalInput").ap()
    wv = nc.dram_tensor("wv", [D, VW], F32R, kind="ExternalInput").ap()
    bq = nc.dram_tensor("bq", [512], F32, kind="ExternalInput").ap()
    bk = nc.dram_tensor("bk", [512], F32, kind="ExternalInput").ap()
    bv = nc.dram_tensor("bv", [VW], F32R, kind="ExternalInput").ap()
    wo = nc.dram_tensor("wo", [512, S], F32R, kind="ExternalInput").ap()
    onesd = nc.dram_tensor("onesd", [1, 128], F32R, kind="ExternalInput").ap()
    out = nc.dram_tensor("out", [S, S], F32, kind="ExternalOutput").ap()
    if debug:
        d_q = nc.dram_tensor("d_q", [512, S], F32, kind="ExternalOutput").ap()
        d_k = nc.dram_tensor("d_k", [512, S], F32, kind="ExternalOutput").ap()
        d_v = nc.dram_tensor("d_v", [8 * 128, VW], F32, kind="ExternalOutput").ap()
        d_m = nc.dram_tensor("d_m", [S, S], F32, kind="ExternalOutput").ap()
        d_vals = nc.dram_tensor("d_vals", [512, S], F32, kind="ExternalOutput").ap()
        d_vps = nc.dram_tensor("d_vps", [HAUG, S], F32, kind="ExternalOutput").ap()
        d_rec = nc.dram_tensor("d_rec", [1, S], F32, kind="ExternalOutput").ap()
        d_rc = nc.dram_tensor("d_rc", [1, S], F32, kind="ExternalOutput").ap()
        d_bcs = nc.dram_tensor("d_bcs", [64, S], F32, kind="ExternalOutput").ap()

    with tile.TileContext(nc) as tc:
        with (
            tc.tile_pool(name="persist", bufs=1) as P,
            tc.tile_pool(name="psum", bufs=1, space="PSUM") as PP,
        ):
            # ---- persistent sbuf ----
            qT = [P.tile([128, S], F32R, tag=f"qT{t}", name=f"qT{t}") for t in range(4)]
            kT = [P.tile([128, S], F32R, tag=f"kT{t}", name=f"kT{t}") for t in range(4)]
            vA = [P.tile([128, VW], F32R, tag=f"vA{t}", name=f"vA{t}") for t in range(8)]
            mN = [P.tile([128, S], F32, tag=f"mN{t}", name=f"mN{t}") for t in range(8)]
            bq_t = P.tile([128, 4], F32, tag="bq", name="bq")
            bk_t = P.tile([128, 4], F32, tag="bk", name="bk")
            bv_row = P.tile([1, VW], F32R, tag="bvrow", name="bvrow")
            ones_row = P.tile([1, 128], F32R, tag="ones", name="ones")
            bvb = P.tile([128, VW], F32, tag="bvb", name="bvb")

            nc.sync.dma_start(bq_t[:], bq.rearrange("(t p) -> p t", p=128))
            nc.sync.dma_start(bk_t[:], bk.rearrange("(t p) -> p t", p=128))
            nc.sync.dma_start(bv_row[:], bv[None, :])
            nc.sync.dma_start(ones_row[:], onesd)

            # broadcast bv across 128 partitions via K=1 matmul
            for h2 in range(2):
                cs = slice(h2 * 260, (h2 + 1) * 260)
                pb = PP.tile([128, 260], F32, tag=f"v{h2}", name=f"v{h2}")
                nc.tensor.matmul(pb[:], ones_row[:, 0:128], bv_row[:, cs],
                                 start=True, stop=True)
                nc.scalar.activation(bvb[:, cs], pb[:], AF.Identity)

            # ---- stage 1: QKV projections ----
            for rep in range(krep):
             with tc.tile_pool(name=f"s1_{rep}", bufs=1) as S1:
                xT_t = [S1.tile([128, S], F32R, tag=f"xT{i}", name=f"xT{i}") for i in range(8)]
                wq_t = [S1.tile([128, 512], F32R, tag=f"wq{i}", name=f"wq{i}") for i in range(8)]
                wk_t = [S1.tile([128, 512], F32R, tag=f"wk{i}", name=f"wk{i}") for i in range(8)]
                wv_t = [S1.tile([128, VW], F32R, tag=f"wv{i}", name=f"wv{i}") for i in range(8)]
                for i in range(8):
                    rs = slice(i * 128, (i + 1) * 128)
                    nc.sync.dma_start(xT_t[i][:], xT[rs, :])
                    nc.sync.dma_start(wq_t[i][:], wq[rs, :])
                    nc.sync.dma_start(wk_t[i][:], wk[rs, :])
                    nc.sync.dma_start(wv_t[i][:], wv[rs, :])

                # q/k feature-major: psum[c,s] accumulated over i-tiles
                for t in range(4):
                    cs = slice(t * 128, (t + 1) * 128)
                    for sh in range(2):
                        ss = slice(sh * 512, (sh + 1) * 512)
                        pq = PP.tile([128, 512], F32, tag=f"st{2 * sh + (t & 1)}", name=f"st{2 * sh + (t & 1)}")
                        pk = PP.tile([128, 512], F32, tag=f"v{2 * sh + (t & 1)}", name=f"v{2 * sh + (t & 1)}")
                        for i in range(8):
                            nc.tensor.matmul(pq[:], wq_t[i][:, cs],
                                             xT_t[i][:, ss],
                                             start=(i == 0), stop=(i == 7))
                        for i in range(8):
                            nc.tensor.matmul(pk[:], wk_t[i][:, cs],
                                             xT_t[i][:, ss],
                                             start=(i == 0), stop=(i == 7))
                        # qT = pq + bq (ACT), kT = (pk + bk)*0.125 (DVE)
                        nc.scalar.activation(qT[t][:, ss], pq[:], AF.Identity,
                                             bias=bq_t[:, t:t + 1])
                        with nc.allow_low_precision(reason="f32r matmul feed"):
                            nc.vector.tensor_scalar(kT[t][:, ss], pk[:],
                                                    bk_t[:, t:t + 1], 0.125,
                                                    AluOpType.add,
                                                    AluOpType.mult)

                # V token-major with augmented ones column
                for st in range(8):
                    ts_ = slice(st * 128, (st + 1) * 128)
                    for h2 in range(2):
                        cs = slice(h2 * 260, (h2 + 1) * 260)
                        pv = PP.tile([128, 260], F32,
                                     tag=f"v{2 * h2 + (st & 1)}",
                                     name=f"pv{2 * h2 + (st & 1)}")
                        for i in range(8):
                            nc.tensor.matmul(pv[:], xT_t[i][:, ts_],
                                             wv_t[i][:, cs],
                                             start=(i == 0), stop=(i == 7))
                        with nc.allow_low_precision(reason="f32r matmul feed"):
                            nc.vector.tensor_tensor(vA[st][:, cs], pv[:],
                                                    bvb[:, cs], AluOpType.add)

             # ---- stage 1.5: mask -> f32 additive ----
             nbias = P.tile([128, 1], F32, tag="nbias", name=f"nbias{rep}")
             nc.vector.memset(nbias[:], NEG)
             with tc.tile_pool(name=f"sm_{rep}", bufs=2) as SM:
                for i in range(8):
                    mi = SM.tile([128, S], I32, tag="mi", name="mi")
                    nc.sync.dma_start(mi[:], maskT[i * 128:(i + 1) * 128, :])
                    # (m - 1) * 8e4 : 0 where mask==1, -8e4 where mask==0
                    nc.scalar.activation(mN[i][:], mi[:], AF.Identity,
                                         bias=nbias[:], scale=-NEG)

             if debug:
                for t in range(4):
                    nc.sync.dma_start(d_q[t * 128:(t + 1) * 128, :], qT[t][:].bitcast(F32))
                    nc.sync.dma_start(d_k[t * 128:(t + 1) * 128, :], kT[t][:].bitcast(F32))
                for t in range(8):
                    nc.sync.dma_start(d_v[t * 128:(t + 1) * 128, :], vA[t][:].bitcast(F32))
                    nc.sync.dma_start(d_m[t * 128:(t + 1) * 128, :], mN[t][:])

             # ---- stage 2+3: attention + output projection ----
             with tc.tile_pool(name=f"sao_{rep}", bufs=1) as SA:
                vals = [SA.tile([128, S], F32R, tag=f"vals{p}", name=f"vals{p}")
                        for p in range(4)]
                wo_t = [SA.tile([128, S], F32R, tag=f"wo{t}", name=f"wo{t}") for t in range(4)]
                for t in range(4):
                    nc.sync.dma_start(wo_t[t][:], wo[t * 128:(t + 1) * 128, :])

                for p in range(4):   # head pair
                    vps = [[PP.tile([HAUG, 512], F32, tag=f"v{2 * hh + qh}", name=f"v{2 * hh + qh}")
                            for qh in range(2)] for hh in range(2)]
                    for kt in range(8):
                        ks = slice(kt * 128, (kt + 1) * 128)
                        for qh in range(2):
                            qs = slice(qh * 512, (qh + 1) * 512)
                            for hh in range(2):
                                h = 2 * p + hh
                                ds = slice(hh * 64, (hh + 1) * 64)
                                stp = PP.tile([128, 512], F32,
                                              tag=f"st{2 * hh + qh}",
                                              name=f"stp{2 * hh + qh}")
                                nc.tensor.matmul(stp[:], kT[p][ds, ks],
                                                 qT[p][ds, qs],
                                                 start=True, stop=True,
                                                 tile_position=(hh * 64, 0))
                                nc.vector.tensor_tensor(stp[:], stp[:],
                                                        mN[kt][:, qs],
                                                        AluOpType.add)
                                pt = SA.tile([128, 512], F32R, tag="pt", name="pt")
                                nc.scalar.activation(pt[:], stp[:], AF.Exp)
                                nc.tensor.matmul(
                                    vps[hh][qh][:],
                                    vA[kt][:, h * HAUG:(h + 1) * HAUG],
                                    pt[:],
                                    start=(kt == 0), stop=(kt == 7))
                    if debug and p == 3:
                        for qh in range(2):
                            dvs = SA.tile([HAUG, 512], F32, tag="dvs", name="dvs")
                            nc.scalar.activation(dvs[:], vps[0][qh][:], AF.Identity)
                            nc.sync.dma_start(d_vps[:, qh * 512:(qh + 1) * 512], dvs[:])
                    # normalize: vals[p][hh*64:(hh+1)*64] = valsT / denom
                    for hh in range(2):
                        den = SA.tile([1, S], F32, tag="den", name="den")
                        rec = SA.tile([1, S], F32, tag="rec", name="rec")
                        scr = SA.tile([1, S], F32, tag="scr", name="scr")
                        rc = SA.tile([1, S], F32R, tag="rc", name="rc")
                        for qh in range(2):
                            qs = slice(qh * 512, (qh + 1) * 512)
                            nc.scalar.activation(den[:, qs],
                                                 vps[hh][qh][64:65, :],
                                                 AF.Identity)
                        nc.vector.reciprocal_approx_accurate(
                            rec[:], den[:], scr[:])
                        with nc.allow_low_precision(reason="f32r matmul feed"):
                            nc.vector.tensor_copy(rc[:], rec[:])
                        if debug and p == 3 and hh == 0:
                            nc.sync.dma_start(d_rec, rec[:])
                            nc.sync.dma_start(d_rc, rc[:].bitcast(F32))
                        for qh in range(2):
                            qs = slice(qh * 512, (qh + 1) * 512)
                            bcp = PP.tile([64, 512], F32, tag=f"st{2 * hh}", name=f"st{2 * hh}")
                            nc.tensor.matmul(bcp[:], ones_row[:, 0:64],
                                             rc[:, qs], start=True, stop=True)
                            bcs = SA.tile([64, 512], F32, tag="bcs", name="bcs")
                            nc.scalar.activation(bcs[:], bcp[:], AF.Identity)
                            if debug and p == 3 and hh == 0:
                                nc.sync.dma_start(
                                    d_bcs[:, qh * 512:(qh + 1) * 512], bcs[:])
                            with nc.allow_low_precision(reason="f32r feed"):
                                nc.vector.tensor_tensor(
                                    vals[p][hh * 64:(hh + 1) * 64, qs],
                                    vps[hh][qh][0:64, :], bcs[:],
                                    AluOpType.mult)

                if debug:
                    for pi in range(4):
                        nc.sync.dma_start(d_vals[pi * 128:(pi + 1) * 128, :],
                                          vals[pi][:].bitcast(F32))

                # output projection: out[q,n] = vals.T @ wo
                for qt in range(8):
                    qs = slice(qt * 128, (qt + 1) * 128)
                    for nh in range(2):
                        ns = slice(nh * 512, (nh + 1) * 512)
                        po = PP.tile([128, 512], F32,
                                     tag=f"st{2 * nh + (qt & 1)}",
                                     name=f"po{2 * nh + (qt & 1)}")
                        for pi in range(4):
                            nc.tensor.matmul(po[:], vals[pi][:, qs],
                                             wo_t[pi][:, ns],
                                             start=(pi == 0), stop=(pi == 3))
                        ot = SA.tile([128, 512], F32, tag="ot", name="ot")
                        nc.scalar.activation(ot[:], po[:], AF.Identity)
                        nc.sync.dma_start(out[qs, ns], ot[:])

    nc.compile()
    return nc


_NC_CACHE = {}


def _get_nc():
    if "nc" not in _NC_CACHE:
        _NC_CACHE["nc"] = build_kernel()
    return _NC_CACHE["nc"]


def shard_inputs(x, mask, Wqkv, bqkv, Wo, bo):
    """Per-core input dicts. Layout/slicing only — no arithmetic."""
    x = np.ascontiguousarray(np.asarray(x, dtype=np.float32))
    mask = np.ascontiguousarray(np.asarray(mask, dtype=np.int32))
    Wqkv = np.asarray(Wqkv, dtype=np.float32)
    bqkv = np.asarray(bqkv, dtype=np.float32)
    Wo = np.asarray(Wo, dtype=np.float32)

    Wr = Wqkv.reshape(D, H, 3, HD)
    br = bqkv.reshape(H, 3, HD)
    ones = np.ones((1, 128), dtype=np.float32)
    in_maps = []
    for c in range(NCORES):
        b, g = c // 2, c % 2
        hs = slice(g * HPC, (g + 1) * HPC)
        wv_aug = np.zeros((D, HPC, HAUG), dtype=np.float32)
        wv_aug[:, :, :HD] = Wr[:, hs, 2, :]
        bv_aug = np.zeros((HPC, HAUG), dtype=np.float32)
        bv_aug[:, :HD] = br[hs, 2, :]
        bv_aug[:, HD] = 1.0
        in_maps.append({
            "xT": np.ascontiguousarray(x[b].T),
            "maskT": np.ascontiguousarray(mask[b].T),
            "wq": np.ascontiguousarray(Wr[:, hs, 0, :].reshape(D, 512)),
            "wk": np.ascontiguousarray(Wr[:, hs, 1, :].reshape(D, 512)),
            "wv": np.ascontiguousarray(wv_aug.reshape(D, VW)),
            "bq": np.ascontiguousarray(br[hs, 0, :].reshape(512)),
            "bk": np.ascontiguousarray(br[hs, 1, :].reshape(512)),
            "bv": np.ascontiguousarray(bv_aug.reshape(VW)),
            "wo": np.ascontiguousarray(Wo[g * 512:(g + 1) * 512, :]),
            "onesd": ones,
        })
    return in_maps


def combine_outputs(results, bo):
    bo = np.asarray(bo, dtype=np.float32)
    out = np.empty((B, S, D), dtype=np.float32)
    for b in range(B):
        out[b] = results[2 * b]["out"] + results[2 * b + 1]["out"] + bo
    return out


def kernel(x, mask, Wqkv, bqkv, Wo, bo):
    nc = _get_nc()
    in_maps = shard_inputs(x, mask, Wqkv, bqkv, Wo, bo)
    res = bass_utils.run_bass_kernel_spmd(nc, in_maps,
                                          core_ids=list(range(NCORES)))
    return combine_outputs(res.results, bo)
